# revision 3
# baseline (speedup 1.0000x reference)
"""BitLinear (RMSNorm + per-row int8 activation quant + ternary GEMM + dequant)
on 8 Trainium2 NeuronCores.

Sharding: data-parallel over the 16384 (B*S) token rows -- 2048 rows per core,
w replicated. This minimizes HBM traffic (each core reads only its x shard plus
a few passes of w) and avoids duplicating the RMSNorm/quant work.

Math notes:
  - Quantized activations are integers in [-127, 127] and weights are ternary
    {-1, 0, 1}: both exactly representable in bf16, so the GEMM runs on the
    TensorEngine in bf16 with f32 PSUM accumulation with zero rounding error
    (|acc| <= 127*4096 < 2^24).
  - round-half-to-even (jnp.round semantics) is implemented with the
    (v + 1.5*2^23) - 1.5*2^23 trick in f32 (IEEE RNE).
  - x is shipped twice (natural and transposed) so that the row statistics use
    free-dim reductions while the quantized K-major operand is produced without
    any on-chip transposes.

Pipelining: rows are processed in blocks; block b+1's stats/quantization run on
ACT/DVE/DMA underneath block b's GEMM on the TensorEngine, hiding the prologue.
"""

import sys

if "/opt/trn_rl_repo" not in sys.path:
    sys.path.insert(0, "/opt/trn_rl_repo")

from contextlib import ExitStack

import ml_dtypes
import numpy as np

import concourse.bacc as bacc
import concourse.bass as bass
import concourse.mybir as mybir
import concourse.tile as tile
from concourse.bass import ts
from concourse.bass_utils import run_bass_kernel_spmd

F32 = mybir.dt.float32
BF16 = mybir.dt.bfloat16
AX = mybir.AxisListType
OP = mybir.AluOpType
ACTF = mybir.ActivationFunctionType

MAGIC = 12582912.0  # 1.5 * 2**23: (v + MAGIC) - MAGIC == round-to-nearest-even(v)
EPS = 1e-5
N_CORES = 8


def build_bitlinear(
    R,
    K,
    O,
    inv_sw127,
    rms_ones=True,
    o_blk=512,
    blocks=None,
    w_bufs=48,
    xq_bufs=None,
):
    """Single-core program. Inputs: x_nat [R,K] f32, x_t [K,R] f32,
    w_t [K,O] bf16 (pre-transposed [in,out]), optional rms [K] f32.
    Output: out [R,O] f32."""
    if blocks is None:
        blocks = [R]
    assert sum(blocks) == R
    nkc = K // 128
    nob = O // o_blk
    assert R % 128 == 0 and K % 128 == 0 and O % o_blk == 0
    nbc_tot = R // 128
    if xq_bufs is None:
        # two blocks' worth of k-tiles so quant(b+1) never stalls on GEMM(b)
        xq_bufs = 2 * nkc if len(blocks) > 1 else nkc

    nc = bacc.Bacc("TRN2", target_bir_lowering=False, debug=False, num_devices=N_CORES)
    x_nat = nc.declare_dram_parameter("x_nat", [R, K], F32, isOutput=False)
    x_t = nc.declare_dram_parameter("x_t", [K, R], F32, isOutput=False)
    w_t = nc.declare_dram_parameter("w_t", [K, O], BF16, isOutput=False)
    rms = None
    if not rms_ones:
        rms = nc.declare_dram_parameter("rms", [K], F32, isOutput=False)
    out = nc.declare_dram_parameter("out", [R, O], F32, isOutput=True)

    with ExitStack() as ctx:
        tc = ctx.enter_context(tile.TileContext(nc))
        singles = ctx.enter_context(tc.tile_pool(name="singles", bufs=1))
        dpool = ctx.enter_context(tc.tile_pool(name="dpool", bufs=1, space="DRAM"))

        ssum = singles.tile([128, nbc_tot], F32)  # per-row sum(x^2)
        mraw = singles.tile([128, nbc_tot], F32)  # per-row max|x*w|
        dq_all = singles.tile([128, nbc_tot], F32)  # per-row dequant scale
        s_dram = dpool.tile([nbc_tot, 128], F32)  # bounce: quant scale, bs-major

        w_rep = None
        rms_cols = None
        if not rms_ones:
            w_rep = singles.tile([128, K], F32)
            rms_bcast = bass.AP(
                tensor=rms.ap().tensor, offset=rms.ap().offset, ap=[[0, 128], [1, K]]
            )
            nc.sync.dma_start(out=w_rep, in_=rms_bcast)
            rms_cols = singles.tile([128, nkc], F32)
            for kk in range(nkc):
                nc.sync.dma_start(
                    out=rms_cols[:, kk : kk + 1], in_=rms.ap()[ts(kk, 128)]
                )

        # pools shared across row blocks (tag-based slot recycling)
        st1x = ctx.enter_context(tc.tile_pool(name="st1x", bufs=2))
        st1sq = ctx.enter_context(tc.tile_pool(name="st1sq", bufs=1))
        scp = ctx.enter_context(tc.tile_pool(name="scp", bufs=2))
        srp = ctx.enter_context(tc.tile_pool(name="srp", bufs=2))
        st2x = ctx.enter_context(tc.tile_pool(name="st2x", bufs=8))
        st2t = ctx.enter_context(tc.tile_pool(name="st2t", bufs=2))
        xqp = ctx.enter_context(tc.tile_pool(name="xqp", bufs=xq_bufs))
        wp = ctx.enter_context(tc.tile_pool(name="wp", bufs=w_bufs))
        pp = ctx.enter_context(tc.tile_pool(name="pp", bufs=8, space="PSUM"))
        outp = ctx.enter_context(tc.tile_pool(name="outp", bufs=3))

        row0 = 0
        for b, Rb in enumerate(blocks):
            cb0 = row0 // 128
            ncb = Rb // 128

            # ---- stage 1: per-row stats (natural layout, free-dim reductions)
            for ci in range(ncb):
                c = cb0 + ci
                xt_ = st1x.tile([128, K], F32, tag="xt", name=f"xt{c}")
                nc.sync.dma_start(out=xt_, in_=x_nat[ts(c, 128), :])
                sq = st1sq.tile([128, K], F32, tag="sq", name=f"sq{c}")
                nc.scalar.activation(
                    out=sq, in_=xt_, func=ACTF.Square, accum_out=ssum[:, c : c + 1]
                )
                if rms_ones:
                    nc.vector.tensor_reduce(
                        out=mraw[:, c : c + 1],
                        in_=xt_,
                        axis=AX.X,
                        op=OP.max,
                        apply_absolute_value=True,
                    )
                else:
                    p = st1sq.tile([128, K], F32, tag="p", name=f"p{c}")
                    nc.vector.tensor_mul(p, xt_, w_rep)
                    nc.vector.tensor_reduce(
                        out=mraw[:, c : c + 1],
                        in_=p,
                        axis=AX.X,
                        op=OP.max,
                        apply_absolute_value=True,
                    )

            # ---- stage 1b: batched per-row scalar math for this block ----
            cs = slice(cb0, cb0 + ncb)
            a = scp.tile([128, ncb], F32, tag="a", name=f"a{b}")
            nc.vector.tensor_scalar(a, ssum[:, cs], 1.0 / K, EPS, OP.mult, OP.add)
            ysq = scp.tile([128, ncb], F32, tag="ysq", name=f"ysq{b}")
            nc.scalar.activation(out=ysq, in_=a, func=ACTF.Sqrt)
            r0 = scp.tile([128, ncb], F32, tag="r0", name=f"r0{b}")
            nc.vector.reciprocal(r0, ysq)
            t1 = scp.tile([128, ncb], F32, tag="t1", name=f"t1{b}")
            nc.vector.tensor_mul(t1, r0, r0)
            t2 = scp.tile([128, ncb], F32, tag="t2", name=f"t2{b}")
            nc.vector.tensor_mul(t2, t1, a)
            t3 = scp.tile([128, ncb], F32, tag="t3", name=f"t3{b}")
            nc.vector.tensor_scalar(t3, t2, -0.5, 1.5, OP.mult, OP.add)
            rstd = scp.tile([128, ncb], F32, tag="rstd", name=f"rstd{b}")
            nc.vector.tensor_mul(rstd, r0, t3)
            ma = scp.tile([128, ncb], F32, tag="ma", name=f"ma{b}")
            nc.vector.tensor_mul(ma, mraw[:, cs], rstd)
            mac = scp.tile([128, ncb], F32, tag="mac", name=f"mac{b}")
            nc.vector.tensor_scalar(mac, ma, 1e-5, None, OP.max)
            nc.vector.tensor_scalar_mul(dq_all[:, cs], mac, inv_sw127)
            inv = scp.tile([128, ncb], F32, tag="inv", name=f"inv{b}")
            nc.vector.reciprocal(inv, mac)
            sc0 = scp.tile([128, ncb], F32, tag="sc0", name=f"sc0{b}")
            nc.vector.tensor_mul(sc0, inv, rstd)
            s_col = scp.tile([128, ncb], F32, tag="s_col", name=f"s_col{b}")
            nc.vector.tensor_scalar_mul(s_col, sc0, 127.0)

            # scatter-transpose s_col -> s_dram rows [cb0, cb0+ncb)
            s_dram_t = bass.AP(
                tensor=s_dram.tensor,
                offset=s_dram.offset + cb0 * 128,
                ap=[[1, 128], [128, ncb]],
            )
            nc.sync.dma_start(out=s_dram_t, in_=s_col)
            # broadcast-read back: s_rep[p, j] = s[row0 + j] for all partitions
            s_rep = srp.tile([128, Rb], F32, tag="srep", name=f"srep{b}")
            s_bcast = bass.AP(
                tensor=s_dram.tensor,
                offset=s_dram.offset + cb0 * 128,
                ap=[[0, 128], [1, Rb]],
            )
            nc.sync.dma_start(out=s_rep, in_=s_bcast)

            # ---- stage 2: quantize (transposed layout) -> xq (bf16, K-major)
            xq_list = []
            for kk in range(nkc):
                xtt = st2x.tile([128, Rb], F32, tag="xtt", name=f"xtt{b}_{kk}")
                nc.sync.dma_start(
                    out=xtt, in_=x_t[ts(kk, 128), row0 : row0 + Rb]
                )
                t = st2t.tile([128, Rb], F32, tag="t", name=f"t{b}_{kk}")
                nc.vector.tensor_mul(t, xtt, s_rep)
                xq = xqp.tile([128, Rb], BF16, tag="xq", name=f"xq{b}_{kk}")
                if rms_ones:
                    nc.vector.tensor_scalar(xq, t, MAGIC, MAGIC, OP.add, OP.subtract)
                else:
                    t2_ = st2t.tile([128, Rb], F32, tag="t2_", name=f"t2_{b}_{kk}")
                    nc.vector.tensor_scalar(
                        t2_, t, rms_cols[:, kk : kk + 1], MAGIC, OP.mult, OP.add
                    )
                    nc.vector.tensor_scalar(xq, t2_, MAGIC, None, OP.subtract)
                xq_list.append(xq)

            # ---- stage 3: GEMM out[bs, o] = xq.T @ w_t, dequant, store ----
            for ob in range(nob):
                wts = []
                for kk in range(nkc):
                    wt_ = wp.tile(
                        [128, o_blk], BF16, tag="wt", name=f"wt{b}_{ob}_{kk}"
                    )
                    nc.sync.dma_start(
                        out=wt_, in_=w_t[ts(kk, 128), ts(ob, o_blk)]
                    )
                    wts.append(wt_)
                for ci in range(ncb):
                    c = cb0 + ci
                    ps = pp.tile([128, o_blk], F32, tag="ps", name=f"ps{b}_{ob}_{ci}")
                    for kk in range(nkc):
                        nc.tensor.matmul(
                            ps,
                            xq_list[kk][:, ts(ci, 128)],
                            wts[kk],
                            start=(kk == 0),
                            stop=(kk == nkc - 1),
                        )
                    ot = outp.tile([128, o_blk], F32, tag="ot", name=f"ot{b}_{ob}_{ci}")
                    nc.scalar.activation(
                        out=ot, in_=ps, func=ACTF.Copy, scale=dq_all[:, c : c + 1]
                    )
                    nc.sync.dma_start(out=out[ts(c, 128), ts(ob, o_blk)], in_=ot)

            row0 += Rb

    nc.compile()
    return nc


_NC_CACHE = {}
DEFAULT_BLOCKS = (512, 512, 512, 512)


def _get_nc(R, K, O, inv_sw127, rms_ones):
    key = (R, K, O, float(inv_sw127), rms_ones)
    if key not in _NC_CACHE:
        blocks = list(DEFAULT_BLOCKS) if R == sum(DEFAULT_BLOCKS) else [R]
        _NC_CACHE[key] = build_bitlinear(
            R, K, O, inv_sw127, rms_ones=rms_ones, blocks=blocks
        )
    return _NC_CACHE[key]


def make_in_maps(x, rms_weight, w_ternary, scale_w, n_cores=N_CORES):
    """Host-side sharding/layout prep. Returns (in_maps, meta)."""
    x = np.asarray(x, dtype=np.float32)
    rms_weight = np.asarray(rms_weight, dtype=np.float32)
    w_ternary = np.asarray(w_ternary, dtype=np.float32)
    scale_w = np.asarray(scale_w, dtype=np.float32)

    B, S, K = x.shape
    Ofeat = w_ternary.shape[0]
    M = B * S
    assert M % n_cores == 0
    R = M // n_cores

    rms_ones = bool(np.all(rms_weight == np.float32(1.0)))
    sw = np.float32(scale_w.reshape(-1)[0])
    inv_sw127 = float(np.float32(1.0) / (np.float32(127.0) * sw))

    xf = x.reshape(M, K)
    w_t_bf = np.ascontiguousarray(w_ternary.T).astype(ml_dtypes.bfloat16)

    in_maps = []
    for i in range(n_cores):
        xs = np.ascontiguousarray(xf[i * R : (i + 1) * R])
        m = {
            "x_nat": xs,
            "x_t": np.ascontiguousarray(xs.T),
            "w_t": w_t_bf,
        }
        if not rms_ones:
            m["rms"] = np.ascontiguousarray(rms_weight)
        in_maps.append(m)
    meta = dict(B=B, S=S, K=K, O=Ofeat, R=R, rms_ones=rms_ones, inv_sw127=inv_sw127)
    return in_maps, meta


def kernel(x, rms_weight, w_ternary, scale_w):
    in_maps, meta = make_in_maps(x, rms_weight, w_ternary, scale_w)
    nc = _get_nc(meta["R"], meta["K"], meta["O"], meta["inv_sw127"], meta["rms_ones"])
    res = run_bass_kernel_spmd(nc, in_maps, list(range(N_CORES)))
    outs = [np.asarray(res.results[i]["out"]) for i in range(N_CORES)]
    full = np.concatenate(outs, axis=0).reshape(meta["B"], meta["S"], meta["O"])
    return full.astype(np.float32, copy=False)


if __name__ == "__main__":
    rng = np.random.default_rng(0)
    B, S, D = 4, 4096, 4096
    x = rng.standard_normal((B, S, D), dtype=np.float32)
    rms_w = np.ones((D,), np.float32)
    w = (rng.integers(0, 3, size=(D, D)) - 1).astype(np.float32)
    sw = np.array([2.0], np.float32)
    out = kernel(x, rms_w, w, sw)
    print(out.shape, out.dtype)


# revision 9
# speedup vs baseline: 1.1455x; 1.1455x over previous
"""BitLinear (RMSNorm + per-row int8 activation quant + ternary GEMM + dequant)
on 8 Trainium2 NeuronCores.

Sharding: data-parallel over the 16384 (B*S) token rows -- 2048 rows per core,
w replicated. This minimizes HBM traffic (each core reads only its x shard plus
a few passes of w) and avoids duplicating the RMSNorm/quant work.

Math notes:
  - Quantized activations are integers in [-127, 127] and weights are ternary
    {-1, 0, 1}: both exactly representable in bf16, so the GEMM runs on the
    TensorEngine in bf16 with f32 PSUM accumulation with zero rounding error
    (|acc| <= 127*4096 < 2^24).
  - round-half-to-even (jnp.round semantics) is implemented with the
    (v + 1.5*2^23) - 1.5*2^23 trick in f32 (IEEE RNE).
  - x is shipped twice (natural and transposed) so that the row statistics use
    free-dim reductions while the quantized K-major operand is produced without
    any on-chip transposes.

Pipelining: rows are processed in blocks; block b+1's stats/quantization run on
ACT/DVE/DMA underneath block b's GEMM on the TensorEngine, hiding the prologue.
"""

import sys

if "/opt/trn_rl_repo" not in sys.path:
    sys.path.insert(0, "/opt/trn_rl_repo")

from contextlib import ExitStack

import ml_dtypes
import numpy as np

import concourse.bacc as bacc
import concourse.bass as bass
import concourse.mybir as mybir
import concourse.tile as tile
from concourse.bass import ts
from concourse.bass_utils import run_bass_kernel_spmd

F32 = mybir.dt.float32
BF16 = mybir.dt.bfloat16
AX = mybir.AxisListType
OP = mybir.AluOpType
ACTF = mybir.ActivationFunctionType

MAGIC = 12582912.0  # 1.5 * 2**23: (v + MAGIC) - MAGIC == round-to-nearest-even(v)
EPS = 1e-5
N_CORES = 8


def build_bitlinear(
    R,
    K,
    O,
    inv_sw127,
    rms_ones=True,
    o_blk=512,
    blocks=None,
    w_bufs=2,
    xq_bufs=None,
):
    """Single-core program. Inputs: x_nat [R,K] f32, x_t [K,R] f32,
    w_t [K,O] bf16 (pre-transposed [in,out]), optional rms [K] f32.
    Output: out [R,O] f32."""
    if blocks is None:
        blocks = [R]
    assert sum(blocks) == R
    nkc = K // 128
    nob = O // o_blk
    assert R % 128 == 0 and K % 128 == 0 and O % o_blk == 0
    nbc_tot = R // 128
    if xq_bufs is None:
        # two blocks' worth of k-tiles so quant(b+1) never stalls on GEMM(b)
        xq_bufs = 2 * nkc if len(blocks) > 1 else nkc

    nc = bacc.Bacc("TRN2", target_bir_lowering=False, debug=False, num_devices=N_CORES)
    x_nat = nc.declare_dram_parameter("x_nat", [R, K], F32, isOutput=False)
    x_t = nc.declare_dram_parameter("x_t", [K, R], F32, isOutput=False)
    # w pre-tiled on host: w_p[ob, p, kk, j] = w[o=ob*o_blk+j, i=kk*128+p]
    # -> each (ob) block is one contiguous DMA with 32KB/partition lines
    w_p = nc.declare_dram_parameter("w_p", [nob, 128, nkc, o_blk], BF16, isOutput=False)
    rms = None
    if not rms_ones:
        rms = nc.declare_dram_parameter("rms", [K], F32, isOutput=False)
    out = nc.declare_dram_parameter("out", [R, O], F32, isOutput=True)

    with ExitStack() as ctx:
        tc = ctx.enter_context(tile.TileContext(nc))
        singles = ctx.enter_context(tc.tile_pool(name="singles", bufs=1))
        dpool = ctx.enter_context(tc.tile_pool(name="dpool", bufs=1, space="DRAM"))

        ssum = singles.tile([128, nbc_tot], F32)  # per-row sum(x^2)
        mraw = singles.tile([128, nbc_tot], F32)  # per-row max|x*w|
        dq_all = singles.tile([128, nbc_tot], F32)  # per-row dequant scale
        s_dram = dpool.tile([nbc_tot, 128], F32)  # bounce: quant scale, bs-major

        w_rep = None
        rms_cols = None
        if not rms_ones:
            w_rep = singles.tile([128, K], F32)
            rms_bcast = bass.AP(
                tensor=rms.ap().tensor, offset=rms.ap().offset, ap=[[0, 128], [1, K]]
            )
            nc.sync.dma_start(out=w_rep, in_=rms_bcast)
            rms_cols = singles.tile([128, nkc], F32)
            for kk in range(nkc):
                nc.sync.dma_start(
                    out=rms_cols[:, kk : kk + 1], in_=rms.ap()[ts(kk, 128)]
                )

        # pools shared across row blocks (tag-based slot recycling)
        st1x = ctx.enter_context(tc.tile_pool(name="st1x", bufs=2))
        st1sq = ctx.enter_context(tc.tile_pool(name="st1sq", bufs=1))
        scp = ctx.enter_context(tc.tile_pool(name="scp", bufs=2))
        srp = ctx.enter_context(tc.tile_pool(name="srp", bufs=2))
        st2x = ctx.enter_context(tc.tile_pool(name="st2x", bufs=6))
        st2t = ctx.enter_context(tc.tile_pool(name="st2t", bufs=2))
        xqp = ctx.enter_context(tc.tile_pool(name="xqp", bufs=xq_bufs))
        wp = ctx.enter_context(tc.tile_pool(name="wp", bufs=w_bufs))
        pp = ctx.enter_context(tc.tile_pool(name="pp", bufs=8, space="PSUM"))
        outp = ctx.enter_context(tc.tile_pool(name="outp", bufs=3))

        row0 = 0
        for b, Rb in enumerate(blocks):
            cb0 = row0 // 128
            ncb = Rb // 128

            # ---- stage 1: per-row stats (natural layout, free-dim reductions)
            for ci in range(ncb):
                c = cb0 + ci
                xt_ = st1x.tile([128, K], F32, tag="xt", name=f"xt{c}")
                nc.sync.dma_start(out=xt_, in_=x_nat[ts(c, 128), :])
                sq = st1sq.tile([128, K], F32, tag="sq", name=f"sq{c}")
                nc.scalar.activation(
                    out=sq, in_=xt_, func=ACTF.Square, accum_out=ssum[:, c : c + 1]
                )
                if rms_ones:
                    nc.vector.tensor_reduce(
                        out=mraw[:, c : c + 1],
                        in_=xt_,
                        axis=AX.X,
                        op=OP.max,
                        apply_absolute_value=True,
                    )
                else:
                    p = st1sq.tile([128, K], F32, tag="p", name=f"p{c}")
                    nc.vector.tensor_mul(p, xt_, w_rep)
                    nc.vector.tensor_reduce(
                        out=mraw[:, c : c + 1],
                        in_=p,
                        axis=AX.X,
                        op=OP.max,
                        apply_absolute_value=True,
                    )

            # ---- stage 1b: batched per-row scalar math for this block ----
            cs = slice(cb0, cb0 + ncb)
            a = scp.tile([128, ncb], F32, tag="a", name=f"a{b}")
            nc.vector.tensor_scalar(a, ssum[:, cs], 1.0 / K, EPS, OP.mult, OP.add)
            ysq = scp.tile([128, ncb], F32, tag="ysq", name=f"ysq{b}")
            nc.scalar.activation(out=ysq, in_=a, func=ACTF.Sqrt)
            r0 = scp.tile([128, ncb], F32, tag="r0", name=f"r0{b}")
            nc.vector.reciprocal(r0, ysq)
            t1 = scp.tile([128, ncb], F32, tag="t1", name=f"t1{b}")
            nc.vector.tensor_mul(t1, r0, r0)
            t2 = scp.tile([128, ncb], F32, tag="t2", name=f"t2{b}")
            nc.vector.tensor_mul(t2, t1, a)
            t3 = scp.tile([128, ncb], F32, tag="t3", name=f"t3{b}")
            nc.vector.tensor_scalar(t3, t2, -0.5, 1.5, OP.mult, OP.add)
            rstd = scp.tile([128, ncb], F32, tag="rstd", name=f"rstd{b}")
            nc.vector.tensor_mul(rstd, r0, t3)
            ma = scp.tile([128, ncb], F32, tag="ma", name=f"ma{b}")
            nc.vector.tensor_mul(ma, mraw[:, cs], rstd)
            mac = scp.tile([128, ncb], F32, tag="mac", name=f"mac{b}")
            nc.vector.tensor_scalar(mac, ma, 1e-5, None, OP.max)
            nc.vector.tensor_scalar_mul(dq_all[:, cs], mac, inv_sw127)
            inv = scp.tile([128, ncb], F32, tag="inv", name=f"inv{b}")
            nc.vector.reciprocal(inv, mac)
            sc0 = scp.tile([128, ncb], F32, tag="sc0", name=f"sc0{b}")
            nc.vector.tensor_mul(sc0, inv, rstd)
            s_col = scp.tile([128, ncb], F32, tag="s_col", name=f"s_col{b}")
            nc.vector.tensor_scalar_mul(s_col, sc0, 127.0)

            # scatter-transpose s_col -> s_dram rows [cb0, cb0+ncb)
            s_dram_t = bass.AP(
                tensor=s_dram.tensor,
                offset=s_dram.offset + cb0 * 128,
                ap=[[1, 128], [128, ncb]],
            )
            nc.sync.dma_start(out=s_dram_t, in_=s_col)
            # broadcast-read back: s_rep[p, j] = s[row0 + j] for all partitions
            s_rep = srp.tile([128, Rb], F32, tag="srep", name=f"srep{b}")
            s_bcast = bass.AP(
                tensor=s_dram.tensor,
                offset=s_dram.offset + cb0 * 128,
                ap=[[0, 128], [1, Rb]],
            )
            nc.sync.dma_start(out=s_rep, in_=s_bcast)

            # ---- stage 2: quantize (transposed layout) -> xq (bf16, K-major)
            xq_list = []
            for kk in range(nkc):
                xtt = st2x.tile([128, Rb], F32, tag="xtt", name=f"xtt{b}_{kk}")
                nc.sync.dma_start(
                    out=xtt, in_=x_t[ts(kk, 128), row0 : row0 + Rb]
                )
                t = st2t.tile([128, Rb], F32, tag="t", name=f"t{b}_{kk}")
                nc.vector.tensor_mul(t, xtt, s_rep)
                xq = xqp.tile([128, Rb], BF16, tag="xq", name=f"xq{b}_{kk}")
                if rms_ones:
                    nc.vector.tensor_scalar(xq, t, MAGIC, MAGIC, OP.add, OP.subtract)
                else:
                    t2_ = st2t.tile([128, Rb], F32, tag="t2_", name=f"t2_{b}_{kk}")
                    nc.vector.tensor_scalar(
                        t2_, t, rms_cols[:, kk : kk + 1], MAGIC, OP.mult, OP.add
                    )
                    nc.vector.tensor_scalar(xq, t2_, MAGIC, None, OP.subtract)
                xq_list.append(xq)

            # ---- stage 3: GEMM out[bs, o] = xq.T @ w, dequant, store ----
            for ob in range(nob):
                wt_ = wp.tile([128, nkc, o_blk], BF16, tag="wt", name=f"wt{b}_{ob}")
                # one contiguous 4MiB DMA; alternate queues (scalar/gpsimd)
                weng = nc.scalar if ob % 2 == 0 else nc.gpsimd
                weng.dma_start(out=wt_, in_=w_p[ob])
                for ci in range(ncb):
                    c = cb0 + ci
                    ps = pp.tile([128, o_blk], F32, tag="ps", name=f"ps{b}_{ob}_{ci}")
                    for kk in range(nkc):
                        nc.tensor.matmul(
                            ps,
                            xq_list[kk][:, ts(ci, 128)],
                            wt_[:, kk, :],
                            start=(kk == 0),
                            stop=(kk == nkc - 1),
                        )
                    ot = outp.tile([128, o_blk], F32, tag="ot", name=f"ot{b}_{ob}_{ci}")
                    nc.scalar.activation(
                        out=ot, in_=ps, func=ACTF.Copy, scale=dq_all[:, c : c + 1]
                    )
                    nc.sync.dma_start(out=out[ts(c, 128), ts(ob, o_blk)], in_=ot)

            row0 += Rb

    nc.compile()
    return nc


_NC_CACHE = {}
DEFAULT_BLOCKS = (512, 512, 512, 512)


def _get_nc(R, K, O, inv_sw127, rms_ones):
    key = (R, K, O, float(inv_sw127), rms_ones)
    if key not in _NC_CACHE:
        blocks = list(DEFAULT_BLOCKS) if R == sum(DEFAULT_BLOCKS) else [R]
        _NC_CACHE[key] = build_bitlinear(
            R, K, O, inv_sw127, rms_ones=rms_ones, blocks=blocks
        )
    return _NC_CACHE[key]


def make_in_maps(x, rms_weight, w_ternary, scale_w, n_cores=N_CORES):
    """Host-side sharding/layout prep. Returns (in_maps, meta)."""
    x = np.asarray(x, dtype=np.float32)
    rms_weight = np.asarray(rms_weight, dtype=np.float32)
    w_ternary = np.asarray(w_ternary, dtype=np.float32)
    scale_w = np.asarray(scale_w, dtype=np.float32)

    B, S, K = x.shape
    Ofeat = w_ternary.shape[0]
    M = B * S
    assert M % n_cores == 0
    R = M // n_cores

    rms_ones = bool(np.all(rms_weight == np.float32(1.0)))
    sw = np.float32(scale_w.reshape(-1)[0])
    inv_sw127 = float(np.float32(1.0) / (np.float32(127.0) * sw))

    xf = x.reshape(M, K)
    # w_p[ob, p, kk, j] = w[o=ob*o_blk+j, i=kk*128+p]
    o_blk = 512
    nkc = K // 128
    nob = Ofeat // o_blk
    w_p = np.ascontiguousarray(
        w_ternary.T.reshape(nkc, 128, nob, o_blk).transpose(2, 1, 0, 3)
    ).astype(ml_dtypes.bfloat16)

    in_maps = []
    for i in range(n_cores):
        xs = np.ascontiguousarray(xf[i * R : (i + 1) * R])
        m = {
            "x_nat": xs,
            "x_t": np.ascontiguousarray(xs.T),
            "w_p": w_p,
        }
        if not rms_ones:
            m["rms"] = np.ascontiguousarray(rms_weight)
        in_maps.append(m)
    meta = dict(B=B, S=S, K=K, O=Ofeat, R=R, rms_ones=rms_ones, inv_sw127=inv_sw127)
    return in_maps, meta


def kernel(x, rms_weight, w_ternary, scale_w):
    in_maps, meta = make_in_maps(x, rms_weight, w_ternary, scale_w)
    nc = _get_nc(meta["R"], meta["K"], meta["O"], meta["inv_sw127"], meta["rms_ones"])
    res = run_bass_kernel_spmd(nc, in_maps, list(range(N_CORES)))
    outs = [np.asarray(res.results[i]["out"]) for i in range(N_CORES)]
    full = np.concatenate(outs, axis=0).reshape(meta["B"], meta["S"], meta["O"])
    return full.astype(np.float32, copy=False)


if __name__ == "__main__":
    rng = np.random.default_rng(0)
    B, S, D = 4, 4096, 4096
    x = rng.standard_normal((B, S, D), dtype=np.float32)
    rms_w = np.ones((D,), np.float32)
    w = (rng.integers(0, 3, size=(D, D)) - 1).astype(np.float32)
    sw = np.array([2.0], np.float32)
    out = kernel(x, rms_w, w, sw)
    print(out.shape, out.dtype)
